# revision 45
# baseline (speedup 1.0000x reference)
"""Trainium2 Bass kernel for nn_AdultConnectome (gnn_message_passing).

Computes y = A^L @ x for a COO sparse adjacency A (100000 nodes, 3.2M edges),
x [100000, 512] fp32, L = layer_number hops.

Distribution (destination-sharded): 8 NeuronCores; core c owns the OUTPUT
row block [12544*c, 12544*(c+1)) and processes exactly the edges whose
destination falls in its block, so the segment-sum per output row is fully
local (no cross-core reduction). The full h table lives in 4 shared HBM
quarter-tables (addr_space="Shared", 25088 rows each, slab-major layout:
quarter q holds local rows [3136q, 3136(q+1)) of every core), each rebuilt
per hop by ONE slab AllGather of the per-core h_next slices.

Per hop:
  1. dma_gather: per edge e, fetch h[col[e], :] (512 bf16 = 1KB rows) from
     the shared quarter-tables (quarter size 25088 respects the int16
     gather-index limit). Edges are sorted per destination row block by
     (source quarter, source); each (block, quarter) run is padded to a
     uniform Q chunks of 128. Gathers are 4 quarter-major streams of
     1024-idx calls (the SWDGE ring limit) spanning block boundaries, one
     SWDGE queue (Q7 descriptor-gen core pair) per quarter.
  2. Per 128-row destination block, build the block's scatter matrices
     P[e, r] = w[e] * (r == rl[e]) with DVE tensor_tensor ops (iota vs
     stride-0-broadcast rl, then * w), and accumulate
     PSUM[r, f] += P_chunk^T @ G_chunk on TensorE (4*Q matmuls per block).
     Quarter-3 matmuls (and the evict) run DELAY blocks behind quarters
     0-2, so the cross-hop dependency on the last slab AllGather is hidden
     behind queued gather work.
  3. Evict the block to a local h_next [12544, 512] bf16.
  4. Slab AllGathers into the next hop's quarter-tables are issued as soon
     as their 3136 input rows have evicted (mid-hop overlap); skipped after
     the final hop (the last h_next is the core's output block).

The rl/w metadata loads into SBUF once (hop-invariant); gather indices
stream in 8-gather slabs per quarter. All structure is computed host-side
from the actual edge data and baked into the SPMD graph (identical on all
8 cores; per-core differences only in input tensors).

Measured on 8 axon trn2 cores: 11.99 ms (baseline 28.66 ms), max-abs rel
err 4.0e-3. Known dead ends: fp8 h-table (8.49 ms but rel err 3.9e-2 —
each fp8 store adds ~1.7% with no averaging); >1024-idx gathers fault the
SWDGE ring; num_swdge_queues alone doesn't speed descriptor-gen;
single_packet=False and dynamic_dma_scratch_size=32768 are both slightly
worse. Bottleneck at 11.99 ms: DMA engines ~82% busy draining 1KB/edge
random-read gather descriptors (~56 ns/desc/engine), ~452 MB/core/hop.
"""

import numpy as np
import ml_dtypes

import concourse.bass as bass
import concourse.bacc as bacc
import concourse.tile as tile
import concourse.mybir as mybir
from concourse.bass_utils import run_bass_kernel_spmd

BF16 = ml_dtypes.bfloat16

N_CORES = 8
P = 128
N_NODES = 100000
N_FEAT = 512
NB = 12544                 # output rows per core (100352 = 8 * 12544)
NPAD = NB * N_CORES        # 100352
NBL = NB // P              # 98 dest row blocks per core
NQT = 4                    # quarter tables (int16 idx limit)
NQ = NPAD // NQT           # 25088 rows per quarter


SL = NB // NQT             # 3136 rows per core per table slab


def _prep_core(rows, cols, ws, core):
    """Per-core edge preprocessing (destination sharding).

    The shared table is slab-major: global source node col (owned by core
    cb = col // NB, local row v = col % NB) lives in quarter q = v // SL at
    position cb * SL + (v % SL). Quarter q is written by the q-th slab
    AllGather of the producing hop, so gathers of quarter q depend only on
    that AllGather."""
    lo, hi = NB * core, NB * (core + 1)
    m = (rows >= lo) & (rows < hi)
    r = rows[m] - lo
    c = cols[m]
    w = ws[m]
    rb = r >> 7
    cb = c // NB
    v = c - cb * NB
    q = v // SL
    cq = cb * SL + (v - q * SL)
    # sort by (dest block, source quarter, source position)
    order = np.lexsort((cq, q, rb))
    rl = (r[order] & 127).astype(np.int64)
    cq = cq[order].astype(np.int64)
    w = ws[m][order]
    rb = rb[order]
    q = q[order]
    cnt = np.bincount(rb * NQT + q, minlength=NBL * NQT).reshape(NBL, NQT)
    return rl, cq, w, rb, q, cnt


GCH = 8                    # chunks per gather (1024 idx ring limit)


def _n_gath(Q):
    """Gathers per quarter stream (98*Q chunks in 8-chunk gathers)."""
    return (NBL * Q + GCH - 1) // GCH


def _pack_core(rl, cq, w, rb, q, cnt, Q):
    """Pack one core's edges into device arrays (wrapped idx + rl/w meta)."""
    ncht = NBL * NQT * Q
    nslots = ncht * P
    run = Q * P                    # slots per (block, quarter) run
    starts = np.zeros(NBL * NQT, dtype=np.int64)
    starts[1:] = np.cumsum(cnt.reshape(-1))[:-1]
    runid = rb * NQT + q
    j_within = np.arange(len(rl)) - starts[runid]
    slot = runid * run + j_within

    idx_flat = np.zeros(nslots, dtype=np.int16)
    idx_flat[slot] = cq.astype(np.int16)
    chunk = slot // P
    part = slot % P
    rl_arr = np.zeros((P, ncht), dtype=BF16)
    w_arr = np.zeros((P, ncht), dtype=BF16)
    rl_arr[part, chunk] = rl.astype(BF16)
    w_arr[part, chunk] = w.astype(BF16)
    rlw = np.concatenate([rl_arr, w_arr], axis=1)

    # quarter-major gather streams: per quarter, concatenate all 98 blocks'
    # runs (block-pure 128-chunks), pad the stream to whole 1024-idx
    # gathers, and wrap each gather [16, 64] -> replicated [128, 64]
    ngath = _n_gath(Q)
    glen = ngath * GCH * P
    cols = []
    for qq in range(NQT):
        stream = np.zeros(glen, dtype=np.int16)
        # slots of quarter qq, block-major: runid = b*NQT+qq
        src = idx_flat.reshape(NBL, NQT, run)[:, qq, :].reshape(-1)
        stream[:src.size] = src
        wrapped = stream.reshape(ngath, GCH * P // 16, 16) \
            .transpose(0, 2, 1)                      # [ngath, 16, 512/8...]
        wrapped = np.tile(wrapped, (1, 8, 1))        # [ngath, 128, 64]
        cols.append(np.concatenate(wrapped, axis=1))  # [128, ngath*64]
    idx_dev = np.concatenate(cols, axis=1)
    assert idx_dev.shape == (P, NQT * ngath * GCH * P // 16)
    return {
        "gidx": np.ascontiguousarray(idx_dev),
        "rlw": np.ascontiguousarray(rlw),
    }


def _build_graph(n_hops, Q):
    """Build the SPMD Bass graph (identical for all cores)."""
    ncht = NBL * NQT * Q
    ngath = _n_gath(Q)
    nicols = NQT * ngath * GCH * P // 16      # total idx cols
    IG = GCH * P // 16                        # idx cols per gather (64)
    IXSLAB = 8                                # gathers per idx-slab load

    nc = bacc.Bacc("TRN2", target_bir_lowering=False, debug=False,
                   num_devices=N_CORES, num_swdge_queues=4)

    h0_in = nc.dram_tensor("h0", [NB, N_FEAT], mybir.dt.bfloat16,
                           kind="ExternalInput")
    gidx_in = nc.dram_tensor("gidx", [P, nicols], mybir.dt.int16,
                             kind="ExternalInput")
    rlw_in = nc.dram_tensor("rlw", [P, 2 * ncht], mybir.dt.bfloat16,
                            kind="ExternalInput")
    y_out = nc.dram_tensor("y", [NB, N_FEAT], mybir.dt.bfloat16,
                           kind="ExternalOutput")

    with tile.TileContext(nc) as tc:
        DELAY = 4
        with tc.tile_pool(name="sbuf_g", bufs=3) as sbuf_g, \
             tc.tile_pool(name="sbuf_p", bufs=2) as sbuf_p, \
             tc.tile_pool(name="sbuf_p3", bufs=DELAY + 2) as sbuf_p3, \
             tc.tile_pool(name="sbuf_e", bufs=4) as sbuf_e, \
             tc.tile_pool(name="sbuf_i", bufs=2) as sbuf_i, \
             tc.tile_pool(name="sbuf_c", bufs=1) as sbuf_c, \
             tc.tile_pool(name="psum", bufs=8, space="PSUM") as psum, \
             tc.tile_pool(name="dram", bufs=2, space="DRAM") as dram:

            # hop-invariant SBUF state
            rlw_sb = sbuf_c.tile([P, 2 * ncht], mybir.dt.bfloat16, tag="rlw")
            nc.sync.dma_start(rlw_sb[:], rlw_in.ap()[:, :])
            iota_f = sbuf_c.tile([P, NQT * Q, P], mybir.dt.bfloat16,
                                 tag="iof")
            nc.gpsimd.iota(iota_f[:], pattern=[[0, NQT * Q], [1, P]],
                           base=0, channel_multiplier=0,
                           allow_small_or_imprecise_dtypes=True)

            def new_tab():
                return [dram.tile([NQ, N_FEAT], mybir.dt.bfloat16,
                                  tag=f"tabq{qq}", addr_space="Shared",
                                  name=f"tabq{qq}")
                        for qq in range(NQT)]

            def slab_ag(src, dst, qq):
                nc.gpsimd.collective_compute(
                    "AllGather", mybir.AluOpType.bypass,
                    replica_groups=[list(range(N_CORES))],
                    ins=[src[SL * qq:SL * (qq + 1), :].opt()],
                    outs=[dst[qq][:, :].opt()],
                )

            # initial shared table from the per-core input blocks
            h0_loc = dram.tile([NB, N_FEAT], mybir.dt.bfloat16, tag="h0l",
                               bufs=1)
            nc.sync.dma_start(h0_loc[:, :], h0_in.ap()[:, :])
            tab = new_tab()
            for qq in range(NQT):
                slab_ag(h0_loc, tab, qq)

            # evict-completion block after which slab qq's AllGather may be
            # issued (+slack so the gpsimd engine doesn't stall on evicts)
            ag_trigger = [(SL * (qq + 1) + P - 1) // P - 1 + 4
                          for qq in range(NQT)]

            for hop in range(n_hops):
                hnl = dram.tile([NB, N_FEAT], mybir.dt.bfloat16, tag="hnl")
                if hop < n_hops - 1:
                    tab_next = new_tab()
                    ag_done = 0
                else:
                    tab_next = None
                    ag_done = NQT

                g_tiles = [[None] * ngath for _ in range(NQT)]
                ix_cur = [None] * NQT
                g_issued = [0] * NQT

                def issue_gathers(upto, quarters, tab=tab, g_tiles=g_tiles,
                                  ix_cur=ix_cur, g_issued=g_issued):
                    upto = min(upto, ngath)
                    for qq in quarters:
                        while g_issued[qq] < upto:
                            gi = g_issued[qq]
                            if gi % IXSLAB == 0:
                                si = gi // IXSLAB
                                nsl = min(IXSLAB, ngath - si * IXSLAB)
                                ix = sbuf_i.tile([P, IXSLAB * IG],
                                                 mybir.dt.int16,
                                                 tag=f"ix{qq}")
                                c0 = (qq * ngath + si * IXSLAB) * IG
                                nc.scalar.dma_start(
                                    ix[:, :nsl * IG],
                                    gidx_in.ap()[:, c0:c0 + nsl * IG])
                                ix_cur[qq] = ix
                            g_t = sbuf_g.tile([P, GCH, N_FEAT],
                                              mybir.dt.bfloat16,
                                              tag=f"g{qq}")
                            io = (gi % IXSLAB) * IG
                            nc.gpsimd.dma_gather(
                                out_ap=g_t[:],
                                in_ap=tab[qq][:, :],
                                idxs_ap=ix_cur[qq][:, io:io + IG],
                                num_idxs=GCH * P,
                                num_idxs_reg=GCH * P,
                                elem_size=N_FEAT,
                                queue_num=qq,
                            )
                            g_tiles[qq][gi] = g_t
                            g_issued[qq] += 1

                # software pipeline: quarter-3 matmuls (and evicts) run
                # DELAY blocks behind quarters 0-2, so the cross-hop stall
                # on the last slab AllGather is hidden behind gather work
                Q3 = (NQT - 1) * Q
                ps_live = {}
                p3_live = {}
                for stage in range(NBL + DELAY):
                    if stage >= DELAY - 1:
                        issue_gathers(
                            ((stage + 2 - DELAY) * Q + GCH - 1) // GCH, (3,))
                    if stage < NBL:
                        b = stage
                        issue_gathers(((b + 1) * Q + GCH - 1) // GCH,
                                      (0, 1, 2))

                        k0 = b * NQT * Q
                        p012 = sbuf_p.tile([P, Q3, P], mybir.dt.bfloat16,
                                           tag="p012", name="p012")
                        rl_b = rlw_sb[:, k0:k0 + Q3].unsqueeze(2) \
                            .broadcast_to([P, Q3, P])
                        w_b = rlw_sb[:, ncht + k0:ncht + k0 + Q3] \
                            .unsqueeze(2).broadcast_to([P, Q3, P])
                        nc.vector.tensor_tensor(
                            out=p012[:], in0=iota_f[:, 0:Q3, :], in1=rl_b,
                            op=mybir.AluOpType.is_equal)
                        nc.vector.tensor_tensor(
                            out=p012[:], in0=p012[:], in1=w_b,
                            op=mybir.AluOpType.mult)
                        p3 = sbuf_p3.tile([P, Q, P], mybir.dt.bfloat16,
                                          tag="p3", name="p3")
                        rl_3 = rlw_sb[:, k0 + Q3:k0 + NQT * Q].unsqueeze(2) \
                            .broadcast_to([P, Q, P])
                        w_3 = rlw_sb[:, ncht + k0 + Q3:ncht + k0 + NQT * Q] \
                            .unsqueeze(2).broadcast_to([P, Q, P])
                        nc.vector.tensor_tensor(
                            out=p3[:], in0=iota_f[:, 0:Q, :], in1=rl_3,
                            op=mybir.AluOpType.is_equal)
                        nc.vector.tensor_tensor(
                            out=p3[:], in0=p3[:], in1=w_3,
                            op=mybir.AluOpType.mult)
                        p3_live[b] = p3

                        ps = psum.tile([P, N_FEAT], mybir.dt.float32,
                                       space="PSUM", tag="ps")
                        ps_live[b] = ps
                        k = 0
                        for qq in range(NQT - 1):
                            for cc in range(Q):
                                j = b * Q + cc
                                nc.tensor.matmul(
                                    out=ps[:],
                                    lhsT=p012[:, k, :],
                                    rhs=g_tiles[qq][j // GCH][:, j % GCH, :],
                                    start=(k == 0),
                                    stop=False,
                                )
                                k += 1

                    if stage >= DELAY:
                        bb = stage - DELAY
                        ps = ps_live.pop(bb)
                        p3 = p3_live.pop(bb)
                        for cc in range(Q):
                            j = bb * Q + cc
                            nc.tensor.matmul(
                                out=ps[:],
                                lhsT=p3[:, cc, :],
                                rhs=g_tiles[3][j // GCH][:, j % GCH, :],
                                start=False,
                                stop=(cc == Q - 1),
                            )
                        ev = sbuf_e.tile([P, N_FEAT], mybir.dt.bfloat16,
                                         tag="evict")
                        nc.vector.tensor_copy(ev[:], ps[:])
                        nc.sync.dma_start(hnl[bb * P:(bb + 1) * P, :], ev[:])

                        while ag_done < NQT and bb >= ag_trigger[ag_done]:
                            slab_ag(hnl, tab_next, ag_done)
                            ag_done += 1

                while ag_done < NQT:
                    slab_ag(hnl, tab_next, ag_done)
                    ag_done += 1
                if tab_next is not None:
                    tab = tab_next

            nc.sync.dma_start(y_out.ap()[:, :], hnl[:, :])

    nc.compile()
    return nc


_GRAPH_CACHE = {}


def kernel(x, weights, row, col, layer_number):
    x = np.asarray(x)
    weights = np.asarray(weights)
    rows = np.asarray(row).astype(np.int64)
    cols = np.asarray(col).astype(np.int64)
    n_hops = int(layer_number)
    if n_hops == 0:
        return x.astype(np.float32)

    preps = [_prep_core(rows, cols, weights, c) for c in range(N_CORES)]
    Q = max(int(np.ceil(p[5].max() / P)) for p in preps)
    Q = max(Q, 1)

    key = (n_hops, Q)
    if key not in _GRAPH_CACHE:
        _GRAPH_CACHE[key] = _build_graph(n_hops, Q)
    nc = _GRAPH_CACHE[key]

    x_pad = np.zeros((NPAD, N_FEAT), dtype=np.float32)
    x_pad[:N_NODES] = x
    x_bf = x_pad.astype(BF16)

    in_maps = []
    for c in range(N_CORES):
        dev = _pack_core(*preps[c], Q)
        in_maps.append({
            "h0": np.ascontiguousarray(x_bf[NB * c:NB * (c + 1)]),
            "gidx": dev["gidx"],
            "rlw": dev["rlw"],
        })

    res = run_bass_kernel_spmd(nc, in_maps, core_ids=list(range(N_CORES)))
    y = np.concatenate([res.results[c]["y"].astype(np.float32)
                        for c in range(N_CORES)], axis=0)
    return y[:N_NODES]


# revision 50
# speedup vs baseline: 1.0083x; 1.0083x over previous
"""Trainium2 Bass kernel for nn_AdultConnectome (gnn_message_passing).

Computes y = A^L @ x for a COO sparse adjacency A (100000 nodes, 3.2M edges),
x [100000, 512] fp32, L = layer_number hops.

Distribution (destination-sharded): 8 NeuronCores; core c owns the OUTPUT
row block [12544*c, 12544*(c+1)) and processes exactly the edges whose
destination falls in its block, so the segment-sum per output row is fully
local (no cross-core reduction). The full h table lives in 4 shared HBM
quarter-tables (addr_space="Shared", 25088 rows each, slab-major layout:
quarter q holds local rows [3136q, 3136(q+1)) of every core), each rebuilt
per hop by ONE slab AllGather of the per-core h_next slices.

Per hop:
  1. dma_gather: per edge e, fetch h[col[e], :] (512 bf16 = 1KB rows) from
     the shared quarter-tables (quarter size 25088 respects the int16
     gather-index limit). Edges are sorted per destination row block by
     (source quarter, source); each (block, quarter) run is padded to a
     uniform Q chunks of 128. Gathers are 4 quarter-major streams of
     1024-idx calls (the SWDGE ring limit) spanning block boundaries, one
     SWDGE queue (Q7 descriptor-gen core pair) per quarter.
  2. Per 128-row destination block, build the block's scatter matrices
     P[e, r] = w[e] * (r == rl[e]) with DVE tensor_tensor ops (iota vs
     stride-0-broadcast rl, then * w), and accumulate
     PSUM[r, f] += P_chunk^T @ G_chunk on TensorE (4*Q matmuls per block).
     Quarter-3 matmuls (and the evict) run DELAY blocks behind quarters
     0-2, so the cross-hop dependency on the last slab AllGather is hidden
     behind queued gather work.
  3. Evict the block to a local h_next [12544, 512] bf16.
  4. Slab AllGathers into the next hop's quarter-tables are issued as soon
     as their 3136 input rows have evicted (mid-hop overlap); skipped after
     the final hop (the last h_next is the core's output block).

The rl/w metadata loads into SBUF once (hop-invariant); gather indices
stream in 8-gather slabs per quarter. All structure is computed host-side
from the actual edge data and baked into the SPMD graph (identical on all
8 cores; per-core differences only in input tensors).

Measured on 8 axon trn2 cores: 11.99 ms (baseline 28.66 ms), max-abs rel
err 4.0e-3. Known dead ends: fp8 h-table (8.49 ms but rel err 3.9e-2 —
each fp8 store adds ~1.7% with no averaging); >1024-idx gathers fault the
SWDGE ring; num_swdge_queues alone doesn't speed descriptor-gen;
single_packet=False and dynamic_dma_scratch_size=32768 are both slightly
worse. Bottleneck at 11.99 ms: DMA engines ~82% busy draining 1KB/edge
random-read gather descriptors (~56 ns/desc/engine), ~452 MB/core/hop.
"""

import numpy as np
import ml_dtypes

import concourse.bass as bass
import concourse.bacc as bacc
import concourse.tile as tile
import concourse.mybir as mybir
from concourse.bass_utils import run_bass_kernel_spmd

BF16 = ml_dtypes.bfloat16

N_CORES = 8
P = 128
N_NODES = 100000
N_FEAT = 512
NB = 12544                 # output rows per core (100352 = 8 * 12544)
NPAD = NB * N_CORES        # 100352
NBL = NB // P              # 98 dest row blocks per core
NQT = 4                    # quarter tables (int16 idx limit)
NQ = NPAD // NQT           # 25088 rows per quarter


SL = NB // NQT             # 3136 rows per core per table slab


def _prep_core(rows, cols, ws, core):
    """Per-core edge preprocessing (destination sharding).

    The shared table is slab-major: global source node col (owned by core
    cb = col // NB, local row v = col % NB) lives in quarter q = v // SL at
    position cb * SL + (v % SL). Quarter q is written by the q-th slab
    AllGather of the producing hop, so gathers of quarter q depend only on
    that AllGather."""
    lo, hi = NB * core, NB * (core + 1)
    m = (rows >= lo) & (rows < hi)
    r = rows[m] - lo
    c = cols[m]
    w = ws[m]
    rb = r >> 7
    cb = c // NB
    v = c - cb * NB
    q = v // SL
    cq = cb * SL + (v - q * SL)
    # sort by (dest block, source quarter, source position)
    order = np.lexsort((cq, q, rb))
    rl = (r[order] & 127).astype(np.int64)
    cq = cq[order].astype(np.int64)
    w = ws[m][order]
    rb = rb[order]
    q = q[order]
    cnt = np.bincount(rb * NQT + q, minlength=NBL * NQT).reshape(NBL, NQT)
    return rl, cq, w, rb, q, cnt


GCH = 8                    # chunks per gather (1024 idx ring limit)


def _n_gath(Q):
    """Gathers per quarter stream (98*Q chunks in 8-chunk gathers)."""
    return (NBL * Q + GCH - 1) // GCH


def _pack_core(rl, cq, w, rb, q, cnt, Q):
    """Pack one core's edges into device arrays (wrapped idx + rl/w meta)."""
    ncht = NBL * NQT * Q
    nslots = ncht * P
    run = Q * P                    # slots per (block, quarter) run
    starts = np.zeros(NBL * NQT, dtype=np.int64)
    starts[1:] = np.cumsum(cnt.reshape(-1))[:-1]
    runid = rb * NQT + q
    j_within = np.arange(len(rl)) - starts[runid]
    slot = runid * run + j_within

    idx_flat = np.zeros(nslots, dtype=np.int16)
    idx_flat[slot] = cq.astype(np.int16)
    chunk = slot // P
    part = slot % P
    rl_arr = np.zeros((P, ncht), dtype=BF16)
    w_arr = np.zeros((P, ncht), dtype=BF16)
    rl_arr[part, chunk] = rl.astype(BF16)
    w_arr[part, chunk] = w.astype(BF16)
    rlw = np.concatenate([rl_arr, w_arr], axis=1)

    # quarter-major gather streams: per quarter, concatenate all 98 blocks'
    # runs (block-pure 128-chunks), pad the stream to whole 1024-idx
    # gathers, and wrap each gather [16, 64] -> replicated [128, 64]
    ngath = _n_gath(Q)
    glen = ngath * GCH * P
    cols = []
    for qq in range(NQT):
        stream = np.zeros(glen, dtype=np.int16)
        # slots of quarter qq, block-major: runid = b*NQT+qq
        src = idx_flat.reshape(NBL, NQT, run)[:, qq, :].reshape(-1)
        stream[:src.size] = src
        wrapped = stream.reshape(ngath, GCH * P // 16, 16) \
            .transpose(0, 2, 1)                      # [ngath, 16, 512/8...]
        wrapped = np.tile(wrapped, (1, 8, 1))        # [ngath, 128, 64]
        cols.append(np.concatenate(wrapped, axis=1))  # [128, ngath*64]
    idx_dev = np.concatenate(cols, axis=1)
    assert idx_dev.shape == (P, NQT * ngath * GCH * P // 16)
    return {
        "gidx": np.ascontiguousarray(idx_dev),
        "rlw": np.ascontiguousarray(rlw),
    }


def _permute_x(x_pad):
    """Slab-major permute the padded input into the 4 quarter tables."""
    node = np.arange(NPAD)
    cb = node // NB
    v = node - cb * NB
    q = v // SL
    pos = q * NQ + cb * SL + (v - q * SL)
    x_perm = np.zeros((NPAD, N_FEAT), dtype=BF16)
    x_perm[pos] = x_pad.astype(BF16)
    return [np.ascontiguousarray(x_perm[qq * NQ:(qq + 1) * NQ])
            for qq in range(NQT)]


def _build_graph(n_hops, Q):
    """Build the SPMD Bass graph (identical for all cores)."""
    ncht = NBL * NQT * Q
    ngath = _n_gath(Q)
    nicols = NQT * ngath * GCH * P // 16      # total idx cols
    IG = GCH * P // 16                        # idx cols per gather (64)
    IXSLAB = 8                                # gathers per idx-slab load

    nc = bacc.Bacc("TRN2", target_bir_lowering=False, debug=False,
                   num_devices=N_CORES, num_swdge_queues=4)

    # hop-0 table: the full slab-major-permuted x, staged per core as four
    # read-only quarter inputs (no startup AllGathers needed)
    qx_in = [nc.dram_tensor(f"qx{qq}", [NQ, N_FEAT], mybir.dt.bfloat16,
                            kind="ExternalInput")
             for qq in range(NQT)]
    gidx_in = nc.dram_tensor("gidx", [P, nicols], mybir.dt.int16,
                             kind="ExternalInput")
    rlw_in = nc.dram_tensor("rlw", [P, 2 * ncht], mybir.dt.bfloat16,
                            kind="ExternalInput")
    y_out = nc.dram_tensor("y", [NB, N_FEAT], mybir.dt.bfloat16,
                           kind="ExternalOutput")

    with tile.TileContext(nc) as tc:
        DELAY = 4
        with tc.tile_pool(name="sbuf_g", bufs=4) as sbuf_g, \
             tc.tile_pool(name="sbuf_p", bufs=2) as sbuf_p, \
             tc.tile_pool(name="sbuf_p3", bufs=DELAY + 2) as sbuf_p3, \
             tc.tile_pool(name="sbuf_e", bufs=4) as sbuf_e, \
             tc.tile_pool(name="sbuf_i", bufs=3) as sbuf_i, \
             tc.tile_pool(name="sbuf_c", bufs=1) as sbuf_c, \
             tc.tile_pool(name="psum", bufs=8, space="PSUM") as psum, \
             tc.tile_pool(name="dram", bufs=2, space="DRAM") as dram:

            # hop-invariant SBUF state
            rlw_sb = sbuf_c.tile([P, 2 * ncht], mybir.dt.bfloat16, tag="rlw")
            nc.sync.dma_start(rlw_sb[:], rlw_in.ap()[:, :])
            iota_f = sbuf_c.tile([P, NQT * Q, P], mybir.dt.bfloat16,
                                 tag="iof")
            nc.gpsimd.iota(iota_f[:], pattern=[[0, NQT * Q], [1, P]],
                           base=0, channel_multiplier=0,
                           allow_small_or_imprecise_dtypes=True)

            def new_tab():
                return [dram.tile([NQ, N_FEAT], mybir.dt.bfloat16,
                                  tag=f"tabq{qq}", addr_space="Shared",
                                  name=f"tabq{qq}")
                        for qq in range(NQT)]

            def slab_ag(src, dst, qq):
                nc.gpsimd.collective_compute(
                    "AllGather", mybir.AluOpType.bypass,
                    replica_groups=[list(range(N_CORES))],
                    ins=[src[SL * qq:SL * (qq + 1), :].opt()],
                    outs=[dst[qq][:, :].opt()],
                )

            tab = [qx_in[qq].ap() for qq in range(NQT)]

            # evict-completion block after which slab qq's AllGather may be
            # issued (+slack so the gpsimd engine doesn't stall on evicts)
            ag_trigger = [(SL * (qq + 1) + P - 1) // P - 1 + 4
                          for qq in range(NQT)]

            for hop in range(n_hops):
                hnl = dram.tile([NB, N_FEAT], mybir.dt.bfloat16, tag="hnl")
                if hop < n_hops - 1:
                    tab_next = new_tab()
                    ag_done = 0
                else:
                    tab_next = None
                    ag_done = NQT

                g_tiles = [[None] * ngath for _ in range(NQT)]
                ix_cur = [None] * NQT
                g_issued = [0] * NQT

                def issue_gathers(upto, quarters, tab=tab, g_tiles=g_tiles,
                                  ix_cur=ix_cur, g_issued=g_issued):
                    upto = min(upto, ngath)
                    for qq in quarters:
                        while g_issued[qq] < upto:
                            gi = g_issued[qq]
                            if gi % IXSLAB == 0:
                                si = gi // IXSLAB
                                nsl = min(IXSLAB, ngath - si * IXSLAB)
                                ix = sbuf_i.tile([P, IXSLAB * IG],
                                                 mybir.dt.int16,
                                                 tag=f"ix{qq}")
                                c0 = (qq * ngath + si * IXSLAB) * IG
                                nc.scalar.dma_start(
                                    ix[:, :nsl * IG],
                                    gidx_in.ap()[:, c0:c0 + nsl * IG])
                                ix_cur[qq] = ix
                            g_t = sbuf_g.tile([P, GCH, N_FEAT],
                                              mybir.dt.bfloat16,
                                              tag=f"g{qq}")
                            io = (gi % IXSLAB) * IG
                            nc.gpsimd.dma_gather(
                                out_ap=g_t[:],
                                in_ap=tab[qq][:, :],
                                idxs_ap=ix_cur[qq][:, io:io + IG],
                                num_idxs=GCH * P,
                                num_idxs_reg=GCH * P,
                                elem_size=N_FEAT,
                                queue_num=qq,
                            )
                            g_tiles[qq][gi] = g_t
                            g_issued[qq] += 1

                # software pipeline: quarter-3 matmuls (and evicts) run
                # DELAY blocks behind quarters 0-2, so the cross-hop stall
                # on the last slab AllGather is hidden behind gather work
                Q3 = (NQT - 1) * Q
                ps_live = {}
                p3_live = {}
                for stage in range(NBL + DELAY):
                    if stage >= DELAY - 1:
                        issue_gathers(
                            ((stage + 2 - DELAY) * Q + GCH - 1) // GCH, (3,))
                    if stage < NBL:
                        b = stage
                        issue_gathers(((b + 1) * Q + GCH - 1) // GCH,
                                      (0, 1, 2))

                        k0 = b * NQT * Q
                        p012 = sbuf_p.tile([P, Q3, P], mybir.dt.bfloat16,
                                           tag="p012", name="p012")
                        rl_b = rlw_sb[:, k0:k0 + Q3].unsqueeze(2) \
                            .broadcast_to([P, Q3, P])
                        w_b = rlw_sb[:, ncht + k0:ncht + k0 + Q3] \
                            .unsqueeze(2).broadcast_to([P, Q3, P])
                        nc.vector.tensor_tensor(
                            out=p012[:], in0=iota_f[:, 0:Q3, :], in1=rl_b,
                            op=mybir.AluOpType.is_equal)
                        nc.vector.tensor_tensor(
                            out=p012[:], in0=p012[:], in1=w_b,
                            op=mybir.AluOpType.mult)
                        p3 = sbuf_p3.tile([P, Q, P], mybir.dt.bfloat16,
                                          tag="p3", name="p3")
                        rl_3 = rlw_sb[:, k0 + Q3:k0 + NQT * Q].unsqueeze(2) \
                            .broadcast_to([P, Q, P])
                        w_3 = rlw_sb[:, ncht + k0 + Q3:ncht + k0 + NQT * Q] \
                            .unsqueeze(2).broadcast_to([P, Q, P])
                        nc.vector.tensor_tensor(
                            out=p3[:], in0=iota_f[:, 0:Q, :], in1=rl_3,
                            op=mybir.AluOpType.is_equal)
                        nc.vector.tensor_tensor(
                            out=p3[:], in0=p3[:], in1=w_3,
                            op=mybir.AluOpType.mult)
                        p3_live[b] = p3

                        ps = psum.tile([P, N_FEAT], mybir.dt.float32,
                                       space="PSUM", tag="ps")
                        ps_live[b] = ps
                        k = 0
                        for qq in range(NQT - 1):
                            for cc in range(Q):
                                j = b * Q + cc
                                nc.tensor.matmul(
                                    out=ps[:],
                                    lhsT=p012[:, k, :],
                                    rhs=g_tiles[qq][j // GCH][:, j % GCH, :],
                                    start=(k == 0),
                                    stop=False,
                                )
                                k += 1

                    if stage >= DELAY:
                        bb = stage - DELAY
                        ps = ps_live.pop(bb)
                        p3 = p3_live.pop(bb)
                        for cc in range(Q):
                            j = bb * Q + cc
                            nc.tensor.matmul(
                                out=ps[:],
                                lhsT=p3[:, cc, :],
                                rhs=g_tiles[3][j // GCH][:, j % GCH, :],
                                start=False,
                                stop=(cc == Q - 1),
                            )
                        ev = sbuf_e.tile([P, N_FEAT], mybir.dt.bfloat16,
                                         tag="evict")
                        nc.vector.tensor_copy(ev[:], ps[:])
                        nc.sync.dma_start(hnl[bb * P:(bb + 1) * P, :], ev[:])

                        while ag_done < NQT and bb >= ag_trigger[ag_done]:
                            slab_ag(hnl, tab_next, ag_done)
                            ag_done += 1

                while ag_done < NQT:
                    slab_ag(hnl, tab_next, ag_done)
                    ag_done += 1
                if tab_next is not None:
                    tab = tab_next

            nc.sync.dma_start(y_out.ap()[:, :], hnl[:, :])

    nc.compile()
    return nc


_GRAPH_CACHE = {}


def kernel(x, weights, row, col, layer_number):
    x = np.asarray(x)
    weights = np.asarray(weights)
    rows = np.asarray(row).astype(np.int64)
    cols = np.asarray(col).astype(np.int64)
    n_hops = int(layer_number)
    if n_hops == 0:
        return x.astype(np.float32)

    preps = [_prep_core(rows, cols, weights, c) for c in range(N_CORES)]
    Q = max(int(np.ceil(p[5].max() / P)) for p in preps)
    Q = max(Q, 1)

    key = (n_hops, Q)
    if key not in _GRAPH_CACHE:
        _GRAPH_CACHE[key] = _build_graph(n_hops, Q)
    nc = _GRAPH_CACHE[key]

    x_pad = np.zeros((NPAD, N_FEAT), dtype=np.float32)
    x_pad[:N_NODES] = x
    qx = _permute_x(x_pad)

    in_maps = []
    for c in range(N_CORES):
        dev = _pack_core(*preps[c], Q)
        im = {"gidx": dev["gidx"], "rlw": dev["rlw"]}
        for qq in range(NQT):
            im[f"qx{qq}"] = qx[qq]
        in_maps.append(im)

    res = run_bass_kernel_spmd(nc, in_maps, core_ids=list(range(N_CORES)))
    y = np.concatenate([res.results[c]["y"].astype(np.float32)
                        for c in range(N_CORES)], axis=0)
    return y[:N_NODES]


# revision 58
# speedup vs baseline: 1.0313x; 1.0228x over previous
"""Trainium2 Bass kernel for nn_AdultConnectome (gnn_message_passing).

Computes y = A^L @ x for a COO sparse adjacency A (100000 nodes, 3.2M edges),
x [100000, 512] fp32, L = layer_number hops.

Distribution (destination-sharded): 8 NeuronCores; core c owns the OUTPUT
row block [12544*c, 12544*(c+1)) and processes exactly the edges whose
destination falls in its block, so the segment-sum per output row is fully
local (no cross-core reduction). The full h table lives in 4 shared HBM
quarter-tables (addr_space="Shared", 25088 rows each, slab-major layout:
quarter q holds local rows [3136q, 3136(q+1)) of every core), each rebuilt
per hop by ONE slab AllGather of the per-core h_next slices.

Per hop:
  1. dma_gather: per edge e, fetch h[col[e], :] (512 bf16 = 1KB rows) from
     the shared quarter-tables (quarter size 25088 respects the int16
     gather-index limit). Edges are sorted per destination row block by
     (source quarter, source); each (block, quarter) run is padded to a
     uniform Q chunks of 128. Gathers are 4 quarter-major streams of
     1024-idx calls (the SWDGE ring limit) spanning block boundaries, one
     SWDGE queue (Q7 descriptor-gen core pair) per quarter.
  2. Per 128-row destination block, build the block's scatter matrices
     P[e, r] = w[e] * (r == rl[e]) with DVE tensor_tensor ops (iota vs
     stride-0-broadcast rl, then * w), and accumulate
     PSUM[r, f] += P_chunk^T @ G_chunk on TensorE (4*Q matmuls per block).
     Quarter-3 matmuls (and the evict) run DELAY blocks behind quarters
     0-2, so the cross-hop dependency on the last slab AllGather is hidden
     behind queued gather work.
  3. Evict the block to a local h_next [12544, 512] bf16.
  4. Slab AllGathers into the next hop's quarter-tables are issued as soon
     as their 3136 input rows have evicted (mid-hop overlap); skipped after
     the final hop (the last h_next is the core's output block).

The rl/w metadata loads into SBUF once (hop-invariant); gather indices
stream in 8-gather slabs per quarter. Hop 0 gathers straight from the
slab-major-permuted x, staged per core as four read-only external quarter
inputs (no startup AllGathers). All structure is computed host-side from
the actual edge data and baked into the SPMD graph (identical on all 8
cores; per-core differences only in input tensors).

Measured on 8 axon trn2 cores: 11.98 ms (baseline 28.66 ms), max-abs rel
err 4.0e-3; run-to-run noise ~±0.07 ms. Known dead ends: fp8 h-table
(8.49 ms but rel err 3.9e-2 — each fp8 store adds ~1.7% with no
averaging); >1024-idx gathers fault the SWDGE ring; num_swdge_queues
alone doesn't speed descriptor-gen; single_packet=False,
dynamic_dma_scratch_size=32768, and DELAY=6 all measure neutral-to-worse.
Bottleneck at 11.98 ms: DMA engines ~80-90% busy draining 1KB/edge
random-read gather descriptors (~56 ns/desc/engine), ~452 MB/core/hop,
plus an inherent ~0.2-0.4 ms/hop tail on the last slab AllGather (its
input rows are the hop's final evicts; q3-delay covers ~40-60 us of it,
PSUM's 8 banks cap the delay depth).
"""

import numpy as np
import ml_dtypes

import concourse.bass as bass
import concourse.bacc as bacc
import concourse.tile as tile
import concourse.mybir as mybir
from concourse.bass_utils import run_bass_kernel_spmd

BF16 = ml_dtypes.bfloat16

N_CORES = 8
P = 128
N_NODES = 100000
N_FEAT = 512
NB = 12544                 # output rows per core (100352 = 8 * 12544)
NPAD = NB * N_CORES        # 100352
NBL = NB // P              # 98 dest row blocks per core
NQT = 4                    # quarter tables (int16 idx limit)
NQ = NPAD // NQT           # 25088 rows per quarter


SL = NB // NQT             # 3136 rows per core per table slab


def _prep_core(rows, cols, ws, core):
    """Per-core edge preprocessing (destination sharding).

    The shared table is slab-major: global source node col (owned by core
    cb = col // NB, local row v = col % NB) lives in quarter q = v // SL at
    position cb * SL + (v % SL). Quarter q is written by the q-th slab
    AllGather of the producing hop, so gathers of quarter q depend only on
    that AllGather."""
    lo, hi = NB * core, NB * (core + 1)
    m = (rows >= lo) & (rows < hi)
    r = rows[m] - lo
    c = cols[m]
    w = ws[m]
    rb = r >> 7
    cb = c // NB
    v = c - cb * NB
    q = v // SL
    cq = cb * SL + (v - q * SL)
    # sort by (dest block, source quarter, source position)
    order = np.lexsort((cq, q, rb))
    rl = (r[order] & 127).astype(np.int64)
    cq = cq[order].astype(np.int64)
    w = ws[m][order]
    rb = rb[order]
    q = q[order]
    cnt = np.bincount(rb * NQT + q, minlength=NBL * NQT).reshape(NBL, NQT)
    return rl, cq, w, rb, q, cnt


GCH = 8                    # chunks per gather (1024 idx ring limit)


def _n_gath(Q):
    """Gathers per quarter stream (98*Q chunks in 8-chunk gathers)."""
    return (NBL * Q + GCH - 1) // GCH


def _pack_core(rl, cq, w, rb, q, cnt, Q):
    """Pack one core's edges into device arrays (wrapped idx + rl/w meta)."""
    ncht = NBL * NQT * Q
    nslots = ncht * P
    run = Q * P                    # slots per (block, quarter) run
    starts = np.zeros(NBL * NQT, dtype=np.int64)
    starts[1:] = np.cumsum(cnt.reshape(-1))[:-1]
    runid = rb * NQT + q
    j_within = np.arange(len(rl)) - starts[runid]
    slot = runid * run + j_within

    idx_flat = np.zeros(nslots, dtype=np.int16)
    idx_flat[slot] = cq.astype(np.int16)
    chunk = slot // P
    part = slot % P
    rl_arr = np.zeros((P, ncht), dtype=BF16)
    w_arr = np.zeros((P, ncht), dtype=BF16)
    rl_arr[part, chunk] = rl.astype(BF16)
    w_arr[part, chunk] = w.astype(BF16)
    rlw = np.concatenate([rl_arr, w_arr], axis=1)

    # quarter-major gather streams: per quarter, concatenate all 98 blocks'
    # runs (block-pure 128-chunks), pad the stream to whole 1024-idx
    # gathers, and wrap each gather [16, 64] -> replicated [128, 64]
    ngath = _n_gath(Q)
    glen = ngath * GCH * P
    cols = []
    for qq in range(NQT):
        stream = np.zeros(glen, dtype=np.int16)
        # slots of quarter qq, block-major: runid = b*NQT+qq
        src = idx_flat.reshape(NBL, NQT, run)[:, qq, :].reshape(-1)
        stream[:src.size] = src
        wrapped = stream.reshape(ngath, GCH * P // 16, 16) \
            .transpose(0, 2, 1)                      # [ngath, 16, 512/8...]
        wrapped = np.tile(wrapped, (1, 8, 1))        # [ngath, 128, 64]
        cols.append(np.concatenate(wrapped, axis=1))  # [128, ngath*64]
    idx_dev = np.concatenate(cols, axis=1)
    assert idx_dev.shape == (P, NQT * ngath * GCH * P // 16)
    return {
        "gidx": np.ascontiguousarray(idx_dev),
        "rlw": np.ascontiguousarray(rlw),
    }


def _permute_x(x_pad):
    """Slab-major permute the padded input into the 4 quarter tables."""
    node = np.arange(NPAD)
    cb = node // NB
    v = node - cb * NB
    q = v // SL
    pos = q * NQ + cb * SL + (v - q * SL)
    x_perm = np.zeros((NPAD, N_FEAT), dtype=BF16)
    x_perm[pos] = x_pad.astype(BF16)
    return [np.ascontiguousarray(x_perm[qq * NQ:(qq + 1) * NQ])
            for qq in range(NQT)]


def _build_graph(n_hops, Q):
    """Build the SPMD Bass graph (identical for all cores)."""
    ncht = NBL * NQT * Q
    ngath = _n_gath(Q)
    nicols = NQT * ngath * GCH * P // 16      # total idx cols
    IG = GCH * P // 16                        # idx cols per gather (64)
    IXSLAB = 8                                # gathers per idx-slab load

    nc = bacc.Bacc("TRN2", target_bir_lowering=False, debug=False,
                   num_devices=N_CORES, num_swdge_queues=4)

    # hop-0 table: the full slab-major-permuted x, staged per core as four
    # read-only quarter inputs (no startup AllGathers needed)
    qx_in = [nc.dram_tensor(f"qx{qq}", [NQ, N_FEAT], mybir.dt.bfloat16,
                            kind="ExternalInput")
             for qq in range(NQT)]
    gidx_in = nc.dram_tensor("gidx", [P, nicols], mybir.dt.int16,
                             kind="ExternalInput")
    rlw_in = nc.dram_tensor("rlw", [P, 2 * ncht], mybir.dt.bfloat16,
                            kind="ExternalInput")
    y_out = nc.dram_tensor("y", [NB, N_FEAT], mybir.dt.bfloat16,
                           kind="ExternalOutput")

    with tile.TileContext(nc) as tc:
        DELAY = 4   # stages quarter-3 matmuls/evicts trail quarters 0-2
        with tc.tile_pool(name="sbuf_g", bufs=4) as sbuf_g, \
             tc.tile_pool(name="sbuf_p", bufs=2) as sbuf_p, \
             tc.tile_pool(name="sbuf_p3", bufs=DELAY + 2) as sbuf_p3, \
             tc.tile_pool(name="sbuf_e", bufs=4) as sbuf_e, \
             tc.tile_pool(name="sbuf_i", bufs=3) as sbuf_i, \
             tc.tile_pool(name="sbuf_c", bufs=1) as sbuf_c, \
             tc.tile_pool(name="psum", bufs=8, space="PSUM") as psum, \
             tc.tile_pool(name="dram", bufs=2, space="DRAM") as dram:

            # hop-invariant SBUF state
            rlw_sb = sbuf_c.tile([P, 2 * ncht], mybir.dt.bfloat16, tag="rlw")
            nc.sync.dma_start(rlw_sb[:], rlw_in.ap()[:, :])
            iota_f = sbuf_c.tile([P, NQT * Q, P], mybir.dt.bfloat16,
                                 tag="iof")
            nc.gpsimd.iota(iota_f[:], pattern=[[0, NQT * Q], [1, P]],
                           base=0, channel_multiplier=0,
                           allow_small_or_imprecise_dtypes=True)

            def new_tab():
                return [dram.tile([NQ, N_FEAT], mybir.dt.bfloat16,
                                  tag=f"tabq{qq}", addr_space="Shared",
                                  name=f"tabq{qq}")
                        for qq in range(NQT)]

            def slab_ag(src, dst, qq):
                nc.gpsimd.collective_compute(
                    "AllGather", mybir.AluOpType.bypass,
                    replica_groups=[list(range(N_CORES))],
                    ins=[src[SL * qq:SL * (qq + 1), :].opt()],
                    outs=[dst[qq][:, :].opt()],
                )

            tab = [qx_in[qq].ap() for qq in range(NQT)]

            # evict-completion block after which slab qq's AllGather may be
            # issued (+slack so the gpsimd engine doesn't stall on evicts)
            ag_trigger = [(SL * (qq + 1) + P - 1) // P - 1 + 4
                          for qq in range(NQT)]

            for hop in range(n_hops):
                if hop < n_hops - 1:
                    hnl = dram.tile([NB, N_FEAT], mybir.dt.bfloat16,
                                    tag="hnl")
                    tab_next = new_tab()
                    ag_done = 0
                else:
                    # last hop: evict straight into the output tensor
                    hnl = y_out.ap()
                    tab_next = None
                    ag_done = NQT

                g_tiles = [[None] * ngath for _ in range(NQT)]
                ix_cur = [None] * NQT
                g_issued = [0] * NQT

                def issue_gathers(upto, quarters, tab=tab, g_tiles=g_tiles,
                                  ix_cur=ix_cur, g_issued=g_issued):
                    upto = min(upto, ngath)
                    for qq in quarters:
                        while g_issued[qq] < upto:
                            gi = g_issued[qq]
                            if gi % IXSLAB == 0:
                                si = gi // IXSLAB
                                nsl = min(IXSLAB, ngath - si * IXSLAB)
                                ix = sbuf_i.tile([P, IXSLAB * IG],
                                                 mybir.dt.int16,
                                                 tag=f"ix{qq}")
                                c0 = (qq * ngath + si * IXSLAB) * IG
                                nc.scalar.dma_start(
                                    ix[:, :nsl * IG],
                                    gidx_in.ap()[:, c0:c0 + nsl * IG])
                                ix_cur[qq] = ix
                            g_t = sbuf_g.tile([P, GCH, N_FEAT],
                                              mybir.dt.bfloat16,
                                              tag=f"g{qq}")
                            io = (gi % IXSLAB) * IG
                            nc.gpsimd.dma_gather(
                                out_ap=g_t[:],
                                in_ap=tab[qq][:, :],
                                idxs_ap=ix_cur[qq][:, io:io + IG],
                                num_idxs=GCH * P,
                                num_idxs_reg=GCH * P,
                                elem_size=N_FEAT,
                                queue_num=qq,
                            )
                            g_tiles[qq][gi] = g_t
                            g_issued[qq] += 1

                # software pipeline: quarter-3 matmuls (and evicts) run
                # DELAY blocks behind quarters 0-2, so the cross-hop stall
                # on the last slab AllGather is hidden behind gather work
                Q3 = (NQT - 1) * Q
                ps_live = {}
                p3_live = {}
                for stage in range(NBL + DELAY):
                    if stage >= DELAY - 1:
                        issue_gathers(
                            ((stage + 2 - DELAY) * Q + GCH - 1) // GCH, (3,))
                    if stage < NBL:
                        b = stage
                        issue_gathers(((b + 1) * Q + GCH - 1) // GCH,
                                      (0, 1, 2))

                        k0 = b * NQT * Q
                        p012 = sbuf_p.tile([P, Q3, P], mybir.dt.bfloat16,
                                           tag="p012", name="p012")
                        rl_b = rlw_sb[:, k0:k0 + Q3].unsqueeze(2) \
                            .broadcast_to([P, Q3, P])
                        w_b = rlw_sb[:, ncht + k0:ncht + k0 + Q3] \
                            .unsqueeze(2).broadcast_to([P, Q3, P])
                        nc.vector.tensor_tensor(
                            out=p012[:], in0=iota_f[:, 0:Q3, :], in1=rl_b,
                            op=mybir.AluOpType.is_equal)
                        nc.vector.tensor_tensor(
                            out=p012[:], in0=p012[:], in1=w_b,
                            op=mybir.AluOpType.mult)
                        p3 = sbuf_p3.tile([P, Q, P], mybir.dt.bfloat16,
                                          tag="p3", name="p3")
                        rl_3 = rlw_sb[:, k0 + Q3:k0 + NQT * Q].unsqueeze(2) \
                            .broadcast_to([P, Q, P])
                        w_3 = rlw_sb[:, ncht + k0 + Q3:ncht + k0 + NQT * Q] \
                            .unsqueeze(2).broadcast_to([P, Q, P])
                        nc.vector.tensor_tensor(
                            out=p3[:], in0=iota_f[:, 0:Q, :], in1=rl_3,
                            op=mybir.AluOpType.is_equal)
                        nc.vector.tensor_tensor(
                            out=p3[:], in0=p3[:], in1=w_3,
                            op=mybir.AluOpType.mult)
                        p3_live[b] = p3

                        ps = psum.tile([P, N_FEAT], mybir.dt.float32,
                                       space="PSUM", tag="ps")
                        ps_live[b] = ps
                        k = 0
                        for qq in range(NQT - 1):
                            for cc in range(Q):
                                j = b * Q + cc
                                nc.tensor.matmul(
                                    out=ps[:],
                                    lhsT=p012[:, k, :],
                                    rhs=g_tiles[qq][j // GCH][:, j % GCH, :],
                                    start=(k == 0),
                                    stop=False,
                                )
                                k += 1

                    if stage >= DELAY:
                        bb = stage - DELAY
                        ps = ps_live.pop(bb)
                        p3 = p3_live.pop(bb)
                        for cc in range(Q):
                            j = bb * Q + cc
                            nc.tensor.matmul(
                                out=ps[:],
                                lhsT=p3[:, cc, :],
                                rhs=g_tiles[3][j // GCH][:, j % GCH, :],
                                start=False,
                                stop=(cc == Q - 1),
                            )
                        ev = sbuf_e.tile([P, N_FEAT], mybir.dt.bfloat16,
                                         tag="evict")
                        nc.vector.tensor_copy(ev[:], ps[:])
                        nc.sync.dma_start(hnl[bb * P:(bb + 1) * P, :], ev[:])

                        while ag_done < NQT and bb >= ag_trigger[ag_done]:
                            slab_ag(hnl, tab_next, ag_done)
                            ag_done += 1

                # remaining quarter-3 gathers must precede the tail
                # AllGathers in gpsimd program order (the AG waits on evicts
                # that need them)
                issue_gathers(ngath, (3,))
                while ag_done < NQT:
                    slab_ag(hnl, tab_next, ag_done)
                    ag_done += 1
                if tab_next is not None:
                    tab = tab_next

    nc.compile()
    return nc


_GRAPH_CACHE = {}


def kernel(x, weights, row, col, layer_number):
    x = np.asarray(x)
    weights = np.asarray(weights)
    rows = np.asarray(row).astype(np.int64)
    cols = np.asarray(col).astype(np.int64)
    n_hops = int(layer_number)
    if n_hops == 0:
        return x.astype(np.float32)

    preps = [_prep_core(rows, cols, weights, c) for c in range(N_CORES)]
    Q = max(int(np.ceil(p[5].max() / P)) for p in preps)
    Q = max(Q, 1)

    key = (n_hops, Q)
    if key not in _GRAPH_CACHE:
        _GRAPH_CACHE[key] = _build_graph(n_hops, Q)
    nc = _GRAPH_CACHE[key]

    x_pad = np.zeros((NPAD, N_FEAT), dtype=np.float32)
    x_pad[:N_NODES] = x
    qx = _permute_x(x_pad)

    in_maps = []
    for c in range(N_CORES):
        dev = _pack_core(*preps[c], Q)
        im = {"gidx": dev["gidx"], "rlw": dev["rlw"]}
        for qq in range(NQT):
            im[f"qx{qq}"] = qx[qq]
        in_maps.append(im)

    res = run_bass_kernel_spmd(nc, in_maps, core_ids=list(range(N_CORES)))
    y = np.concatenate([res.results[c]["y"].astype(np.float32)
                        for c in range(N_CORES)], axis=0)
    return y[:N_NODES]
